# revision 1
# baseline (speedup 1.0000x reference)
"""GalaxyTileDecoder on 8 Trainium2 NeuronCores.

The reference pipeline (linear decode -> zero-pad -> gate -> bilinear
grid_sample -> sum over M=2 sources) collapses algebraically: the sample
grid is a pure per-source translation, sampling the padded 53x53 image at
(y, x) = (i + 2.5 - 4*locs[...,0], j + 2.5 - 4*locs[...,1]).  Folding the
integer shift (one-hot over 6 positions per axis), the bilinear weights,
the decoder bias, the galaxy_bool gate, and the M-sum into an expanded
feature dimension turns the whole forward into a single matmul per ptile
batch:

    out[p, :] = sum_par z_exp[p, par, :] @ W_exp          (K=324/parity)

with W_exp[(a, b, f), (i, j)] = canvas9[f, a+i, b+j] the 6x6 shifted
52x52 windows of the 9 basis images (8 decoder rows + bias) in a 57x57
zero canvas, and z_exp the per-source sparse coefficients
bool * z9[f] * wy[a] * wx[b].  The M=2 partner sum accumulates in PSUM.

Data parallel over the ptile axis: 1250 ptiles per core, no collectives.
"""

import math
import os

import numpy as np

P_TOTAL = 10000
M = 2
N_CORES = 8
PT = P_TOTAL // N_CORES          # ptiles per core
F = 9                            # 8 decoder features + bias
A = 6                            # y-shift positions (-2..3)
B = 6                            # x-shift positions (-2..3)
K = A * B * F                    # 324 expanded features per parity
OUT_HW = 52
COLS = OUT_HW * OUT_HW           # 2704
HALF = COLS // 2                 # 1352
CANVAS = 57

_DT_NAME = os.environ.get("BASS_GAL_DT", "bf16")

_cache = {}


def _build_program(dt_name):
    import concourse.bass as bass
    import concourse.tile as tile
    from concourse import bacc, mybir

    dt_map = {
        "bf16": mybir.dt.bfloat16,
        "f32": mybir.dt.float32,
        "f32r": mybir.dt.float32r,
    }
    DT = dt_map[dt_name]

    nc = bacc.Bacc(trn_type="TRN2")
    zt = nc.dram_tensor("zt", [M, K, PT], DT, kind="ExternalInput")
    wx = nc.dram_tensor("wx", [K, COLS], DT, kind="ExternalInput")
    out = nc.dram_tensor("out", [PT, COLS], mybir.dt.float32, kind="ExternalOutput")

    KCH = [(0, 128), (128, 256), (256, K)]
    SEGS = [(0, 512), (512, 1024), (1024, HALF)]
    n_batches = math.ceil(PT / 128)

    with tile.TileContext(nc) as tc:
        with (
            tc.tile_pool(name="wz", bufs=1) as const_pool,
            tc.tile_pool(name="o", bufs=3) as opool,
            tc.tile_pool(name="ps", bufs=2, space="PSUM") as pspool,
        ):
            w_tiles = []
            for ci, (k0, k1) in enumerate(KCH):
                wt = const_pool.tile([k1 - k0, COLS], DT, tag=f"w{ci}")
                nc.sync.dma_start(wt[:], wx[k0:k1, :])
                w_tiles.append(wt)
            z_tiles = {}
            for par in range(M):
                for ci, (k0, k1) in enumerate(KCH):
                    ztile = const_pool.tile([k1 - k0, PT], DT, tag=f"z{par}_{ci}")
                    nc.sync.dma_start(ztile[:], zt[par, k0:k1, :])
                    z_tiles[par, ci] = ztile

            for bi in range(n_batches):
                b0 = bi * 128
                bs = min(128, PT - b0)
                for h in range(2):
                    c0 = h * HALF
                    ps = pspool.tile([128, HALF], mybir.dt.float32, tag="ps")
                    q = 0
                    nq = M * len(KCH)
                    for par in range(M):
                        for ci in range(len(KCH)):
                            lhsT = z_tiles[par, ci][:, b0:b0 + bs]
                            for (s0, s1) in SEGS:
                                nc.tensor.matmul(
                                    ps[0:bs, s0:s1],
                                    lhsT,
                                    w_tiles[ci][:, c0 + s0:c0 + s1],
                                    start=(q == 0),
                                    stop=(q == nq - 1),
                                )
                            q += 1
                    osb = opool.tile([128, HALF], mybir.dt.float32, tag="osb")
                    nc.vector.tensor_copy(osb[0:bs, :], ps[0:bs, :])
                    nc.sync.dma_start(out[b0:b0 + bs, c0:c0 + HALF], osb[0:bs, :])
    nc.compile()
    return nc


def _get_program(dt_name):
    if dt_name not in _cache:
        _cache[dt_name] = _build_program(dt_name)
    return _cache[dt_name]


def _host_expand(locs, galaxy_params, galaxy_bool, W_dec, b_dec, np_dtype):
    """Build z_expT (M, K, P_TOTAL) and Wexp (K, COLS) on the host."""
    locs = np.asarray(locs, np.float32).reshape(-1, 2)
    params = np.asarray(galaxy_params, np.float32).reshape(-1, 8)
    gbool = np.asarray(galaxy_bool, np.float32).reshape(-1, 1)
    W = np.asarray(W_dec, np.float32)
    b = np.asarray(b_dec, np.float32)
    N = locs.shape[0]

    sy = 2.5 - 4.0 * locs[:, 0]
    sx = 2.5 - 4.0 * locs[:, 1]
    m = np.floor(sy)
    k = np.floor(sx)
    fy = (sy - m).astype(np.float32)
    fx = (sx - k).astype(np.float32)
    m = m.astype(np.int64)
    k = k.astype(np.int64)
    ar = np.arange(N)
    cy = np.zeros((N, A), np.float32)
    cx = np.zeros((N, B), np.float32)
    cy[ar, m + 2] = 1.0 - fy
    cy[ar, m + 3] = fy
    cx[ar, k + 2] = 1.0 - fx
    cx[ar, k + 3] = fx

    z9 = np.concatenate([params, np.ones((N, 1), np.float32)], axis=1) * gbool
    z_exp = (cy[:, :, None, None] * cx[:, None, :, None] * z9[:, None, None, :])
    z_exp = z_exp.reshape(P_TOTAL, M, K)
    # (M, K, P_TOTAL), parity-major so each core slices contiguous ptiles
    z_expT = np.ascontiguousarray(z_exp.transpose(1, 2, 0), dtype=np_dtype)

    canvas9 = np.zeros((F, CANVAS, CANVAS), np.float32)
    canvas9[:8, 3:54, 3:54] = W.reshape(8, 51, 51)
    canvas9[8, 3:54, 3:54] = b.reshape(51, 51)
    sw = np.lib.stride_tricks.sliding_window_view(canvas9, (OUT_HW, OUT_HW), axis=(1, 2))
    Wexp = np.ascontiguousarray(
        sw.transpose(1, 2, 0, 3, 4).reshape(K, COLS), dtype=np_dtype)
    return z_expT, Wexp


def kernel(locs, galaxy_params, galaxy_bool, W_dec, b_dec, _trace=False):
    import ml_dtypes
    from concourse.bass_utils import run_bass_kernel_spmd

    np_dtype = {
        "bf16": ml_dtypes.bfloat16,
        "f32": np.float32,
        "f32r": np.float32,
    }[_DT_NAME]

    z_expT, Wexp = _host_expand(
        locs, galaxy_params, galaxy_bool, W_dec, b_dec, np_dtype)

    nc = _get_program(_DT_NAME)
    in_maps = [
        {
            "zt": np.ascontiguousarray(z_expT[:, :, c * PT:(c + 1) * PT]),
            "wx": Wexp,
        }
        for c in range(N_CORES)
    ]
    kwargs = {}
    if _trace:
        kwargs["trace"] = True
    res = run_bass_kernel_spmd(nc, in_maps, core_ids=list(range(N_CORES)), **kwargs)

    out = np.concatenate([res.results[c]["out"] for c in range(N_CORES)], axis=0)
    out = out.reshape(P_TOTAL, 1, OUT_HW, OUT_HW)
    if _trace:
        kernel._last_result = res
    return out, out


# revision 2
# speedup vs baseline: 1.2018x; 1.2018x over previous
"""GalaxyTileDecoder on 8 Trainium2 NeuronCores.

The reference pipeline (linear decode -> zero-pad -> gate -> bilinear
grid_sample -> sum over M=2 sources) collapses algebraically: the sample
grid is a pure per-source translation, sampling the padded 53x53 image at
(y, x) = (i + 2.5 - 4*locs[...,0], j + 2.5 - 4*locs[...,1]).  Folding the
integer shift (one-hot over 6 positions per axis), the bilinear weights,
the decoder bias, the galaxy_bool gate, and the M-source sum into an
expanded feature dimension turns the whole forward into one matmul:

    out[p, :] = (sum_par z_exp[p, par, :]) @ W_exp        (K=324)

with W_exp[(a, b, f), (i, j)] = canvas9[f, a+i, b+j] the 6x6 shifted
52x52 windows of the 9 basis images (8 decoder rows + bias) in a 57x57
zero canvas, and z_exp the per-source sparse coefficients
bool * z9[f] * wy[a] * wx[b].  The host computes the tiny coefficient
expansion (~0.002% of FLOPs); the device does the 10000x324x2704 matmul.

Data parallel over the ptile axis: 1250 ptiles per core, no collectives.
"""

import math
import os

import numpy as np

P_TOTAL = 10000
M = 2
N_CORES = 8
PT = P_TOTAL // N_CORES          # ptiles per core
F = 9                            # 8 decoder features + bias
A = 6                            # y-shift positions (-2..3)
B = 6                            # x-shift positions (-2..3)
K = A * B * F                    # 324 expanded features
OUT_HW = 52
COLS = OUT_HW * OUT_HW           # 2704
HALF = COLS // 2                 # 1352
CANVAS = 57

_DT_NAME = os.environ.get("BASS_GAL_DT", "bf16")

_cache = {}


def _build_program(dt_name):
    import concourse.bass as bass  # noqa: F401  (registers engines)
    import concourse.tile as tile
    from concourse import bacc, mybir

    dt_map = {
        "bf16": mybir.dt.bfloat16,
        "f32": mybir.dt.float32,
        "f32r": mybir.dt.float32r,
    }
    DT = dt_map[dt_name]

    nc = bacc.Bacc(trn_type="TRN2")
    zt = nc.dram_tensor("zt", [K, PT], DT, kind="ExternalInput")
    wx = nc.dram_tensor("wx", [K, COLS], DT, kind="ExternalInput")
    out = nc.dram_tensor("out", [PT, COLS], mybir.dt.float32, kind="ExternalOutput")

    KCH = [(0, 128), (128, 256), (256, K)]
    SEGS = [(0, 512), (512, 1024), (1024, HALF)]
    n_batches = math.ceil(PT / 128)

    with tile.TileContext(nc) as tc:
        with (
            tc.tile_pool(name="w", bufs=1) as wpool,
            tc.tile_pool(name="z", bufs=4) as zpool,
            tc.tile_pool(name="o", bufs=3) as opool,
            tc.tile_pool(name="ps", bufs=2, space="PSUM") as pspool,
        ):
            # weight halves: h=0 cols land first so batch 0 starts early
            w_tiles = {}
            for h in range(2):
                for ci, (k0, k1) in enumerate(KCH):
                    wt = wpool.tile([k1 - k0, HALF], DT, tag=f"w{ci}_{h}")
                    nc.sync.dma_start(wt[:], wx[k0:k1, h * HALF:(h + 1) * HALF])
                    w_tiles[ci, h] = wt

            for bi in range(n_batches):
                b0 = bi * 128
                bs = min(128, PT - b0)
                z_b = []
                for ci, (k0, k1) in enumerate(KCH):
                    ztile = zpool.tile([k1 - k0, 128], DT, tag=f"z{ci}")
                    nc.sync.dma_start(ztile[:, 0:bs], zt[k0:k1, b0:b0 + bs])
                    z_b.append(ztile)
                for h in range(2):
                    ps = pspool.tile([128, HALF], mybir.dt.float32, tag="ps")
                    for ci in range(len(KCH)):
                        for (s0, s1) in SEGS:
                            nc.tensor.matmul(
                                ps[0:bs, s0:s1],
                                z_b[ci][:, 0:bs],
                                w_tiles[ci, h][:, s0:s1],
                                start=(ci == 0),
                                stop=(ci == len(KCH) - 1),
                            )
                    osb = opool.tile([128, HALF], mybir.dt.float32, tag="osb")
                    nc.vector.tensor_copy(osb[0:bs, :], ps[0:bs, :])
                    nc.sync.dma_start(out[b0:b0 + bs, h * HALF:(h + 1) * HALF],
                                      osb[0:bs, :])
    nc.compile()
    return nc


def _get_program(dt_name):
    if dt_name not in _cache:
        _cache[dt_name] = _build_program(dt_name)
    return _cache[dt_name]


def _host_expand(locs, galaxy_params, galaxy_bool, W_dec, b_dec, np_dtype):
    """Build zt (K, P_TOTAL) parity-summed coefficients and Wexp (K, COLS)."""
    locs = np.asarray(locs, np.float32).reshape(-1, 2)
    params = np.asarray(galaxy_params, np.float32).reshape(-1, 8)
    gbool = np.asarray(galaxy_bool, np.float32).reshape(-1, 1)
    W = np.asarray(W_dec, np.float32)
    b = np.asarray(b_dec, np.float32)
    N = locs.shape[0]

    sy = 2.5 - 4.0 * locs[:, 0]
    sx = 2.5 - 4.0 * locs[:, 1]
    m = np.floor(sy)
    k = np.floor(sx)
    fy = (sy - m).astype(np.float32)
    fx = (sx - k).astype(np.float32)
    m = m.astype(np.int64)
    k = k.astype(np.int64)
    ar = np.arange(N)
    cy = np.zeros((N, A), np.float32)
    cx = np.zeros((N, B), np.float32)
    cy[ar, m + 2] = 1.0 - fy
    cy[ar, m + 3] = fy
    cx[ar, k + 2] = 1.0 - fx
    cx[ar, k + 3] = fx

    z9 = np.concatenate([params, np.ones((N, 1), np.float32)], axis=1) * gbool
    z_exp = (cy[:, :, None, None] * cx[:, None, :, None] * z9[:, None, None, :])
    # sum the M=2 sources of each ptile (matmul is linear in z_exp)
    z_sum = z_exp.reshape(P_TOTAL, M, K).sum(axis=1)
    z_t = np.ascontiguousarray(z_sum.T, dtype=np_dtype)       # (K, P_TOTAL)

    canvas9 = np.zeros((F, CANVAS, CANVAS), np.float32)
    canvas9[:8, 3:54, 3:54] = W.reshape(8, 51, 51)
    canvas9[8, 3:54, 3:54] = b.reshape(51, 51)
    sw = np.lib.stride_tricks.sliding_window_view(canvas9, (OUT_HW, OUT_HW), axis=(1, 2))
    Wexp = np.ascontiguousarray(
        sw.transpose(1, 2, 0, 3, 4).reshape(K, COLS), dtype=np_dtype)
    return z_t, Wexp


def kernel(locs, galaxy_params, galaxy_bool, W_dec, b_dec, _trace=False):
    import ml_dtypes
    from concourse.bass_utils import run_bass_kernel_spmd

    np_dtype = {
        "bf16": ml_dtypes.bfloat16,
        "f32": np.float32,
        "f32r": np.float32,
    }[_DT_NAME]

    z_t, Wexp = _host_expand(
        locs, galaxy_params, galaxy_bool, W_dec, b_dec, np_dtype)

    nc = _get_program(_DT_NAME)
    in_maps = [
        {
            "zt": np.ascontiguousarray(z_t[:, c * PT:(c + 1) * PT]),
            "wx": Wexp,
        }
        for c in range(N_CORES)
    ]
    kwargs = {}
    if _trace:
        kwargs["trace"] = True
    res = run_bass_kernel_spmd(nc, in_maps, core_ids=list(range(N_CORES)), **kwargs)

    out = np.concatenate([res.results[c]["out"] for c in range(N_CORES)], axis=0)
    out = out.reshape(P_TOTAL, 1, OUT_HW, OUT_HW)
    if _trace:
        kernel._last_result = res
    return out, out


# revision 5
# speedup vs baseline: 1.4284x; 1.1886x over previous
"""GalaxyTileDecoder on 8 Trainium2 NeuronCores.

The reference pipeline (linear decode -> zero-pad -> gate -> bilinear
grid_sample -> sum over M=2 sources) collapses algebraically: the sample
grid is a pure per-source translation, sampling the padded 53x53 image at
(y, x) = (i + 2.5 - 4*locs[...,0], j + 2.5 - 4*locs[...,1]).  Folding the
integer shift (one-hot over 6 positions per axis), the bilinear weights,
the decoder bias, the galaxy_bool gate, and the M-source sum into an
expanded feature dimension turns the whole forward into one matmul:

    out[p, :] = (sum_par z_exp[p, par, :]) @ W_exp        (K=324)

with W_exp[(a, b, f), (i, j)] = canvas9[f, a+i, b+j] the 6x6 shifted
52x52 windows of the 9 basis images (8 decoder rows + bias) in a 57x57
zero canvas, and z_exp the per-source sparse coefficients
bool * z9[f] * wy[a] * wx[b].  The host computes the tiny coefficient
expansion (~0.002% of FLOPs); the device does the 10000x324x2704 matmul.

Data parallel over the ptile axis: 1250 ptiles per core, no collectives.
"""

import math
import os

import numpy as np

P_TOTAL = 10000
M = 2
N_CORES = 8
PT = P_TOTAL // N_CORES          # ptiles per core
F = 9                            # 8 decoder features + bias
A = 6                            # y-shift positions (-2..3)
B = 6                            # x-shift positions (-2..3)
K = A * B * F                    # 324 expanded features
OUT_HW = 52
COLS = OUT_HW * OUT_HW           # 2704
HALF = COLS // 2                 # 1352
CANVAS = 57

_DT_NAME = os.environ.get("BASS_GAL_DT", "bf16")

_cache = {}


def _build_program(dt_name):
    import concourse.bass as bass  # noqa: F401  (registers engines)
    import concourse.tile as tile
    from concourse import bacc, mybir

    dt_map = {
        "bf16": mybir.dt.bfloat16,
        "f32": mybir.dt.float32,
        "f32r": mybir.dt.float32r,
    }
    DT = dt_map[dt_name]

    n_batches = math.ceil(PT / 128)
    nc = bacc.Bacc(trn_type="TRN2")
    # host-blocked layouts so every DMA reads a fully contiguous DRAM block
    zt = nc.dram_tensor("zt", [n_batches, K, 128], DT, kind="ExternalInput")
    wx = nc.dram_tensor("wx", [2, K, HALF], DT, kind="ExternalInput")
    out = nc.dram_tensor("out", [PT, COLS], mybir.dt.float32, kind="ExternalOutput")

    KCH = [(0, 128), (128, 256), (256, K)]
    SEGS = [(0, 512), (512, 1024), (1024, HALF)]

    with tile.TileContext(nc) as tc:
        with (
            tc.tile_pool(name="w", bufs=1) as wpool,
            tc.tile_pool(name="z", bufs=4) as zpool,
            tc.tile_pool(name="o", bufs=3) as opool,
            tc.tile_pool(name="ps", bufs=2, space="PSUM") as pspool,
        ):
            # inputs go through the gpsimd DMA queue, outputs through sync:
            # each sequencer issues in program order, so sharing one queue
            # would stall batch b+1 input prefetch behind batch b's output.
            w_tiles = {}
            for h in range(2):
                for ci, (k0, k1) in enumerate(KCH):
                    wt = wpool.tile([k1 - k0, HALF], DT, tag=f"w{ci}_{h}")
                    nc.gpsimd.dma_start(wt[:], wx[h, k0:k1, :])
                    w_tiles[ci, h] = wt

            for bi in range(n_batches):
                b0 = bi * 128
                bs = min(128, PT - b0)
                z_b = []
                for ci, (k0, k1) in enumerate(KCH):
                    ztile = zpool.tile([k1 - k0, 128], DT, tag=f"z{ci}")
                    nc.gpsimd.dma_start(ztile[:], zt[bi, k0:k1, :])
                    z_b.append(ztile)
                for h in range(2):
                    ps = pspool.tile([128, HALF], mybir.dt.float32, tag="ps")
                    for ci in range(len(KCH)):
                        for (s0, s1) in SEGS:
                            nc.tensor.matmul(
                                ps[0:bs, s0:s1],
                                z_b[ci][:, 0:bs],
                                w_tiles[ci, h][:, s0:s1],
                                start=(ci == 0),
                                stop=(ci == len(KCH) - 1),
                            )
                    osb = opool.tile([128, HALF], mybir.dt.float32, tag="osb")
                    nc.vector.tensor_copy(osb[0:bs, :], ps[0:bs, :])
                    nc.sync.dma_start(out[b0:b0 + bs, h * HALF:(h + 1) * HALF],
                                      osb[0:bs, :])
    nc.compile()
    return nc


def _get_program(dt_name):
    if dt_name not in _cache:
        _cache[dt_name] = _build_program(dt_name)
    return _cache[dt_name]


def _host_expand(locs, galaxy_params, galaxy_bool, W_dec, b_dec, np_dtype):
    """Build zt (K, P_TOTAL) parity-summed coefficients and Wexp (K, COLS)."""
    locs = np.asarray(locs, np.float32).reshape(-1, 2)
    params = np.asarray(galaxy_params, np.float32).reshape(-1, 8)
    gbool = np.asarray(galaxy_bool, np.float32).reshape(-1, 1)
    W = np.asarray(W_dec, np.float32)
    b = np.asarray(b_dec, np.float32)
    N = locs.shape[0]

    sy = 2.5 - 4.0 * locs[:, 0]
    sx = 2.5 - 4.0 * locs[:, 1]
    m = np.floor(sy)
    k = np.floor(sx)
    fy = (sy - m).astype(np.float32)
    fx = (sx - k).astype(np.float32)
    m = m.astype(np.int64)
    k = k.astype(np.int64)
    ar = np.arange(N)
    cy = np.zeros((N, A), np.float32)
    cx = np.zeros((N, B), np.float32)
    cy[ar, m + 2] = 1.0 - fy
    cy[ar, m + 3] = fy
    cx[ar, k + 2] = 1.0 - fx
    cx[ar, k + 3] = fx

    z9 = np.concatenate([params, np.ones((N, 1), np.float32)], axis=1) * gbool
    z_exp = (cy[:, :, None, None] * cx[:, None, :, None] * z9[:, None, None, :])
    # sum the M=2 sources of each ptile (matmul is linear in z_exp)
    z_sum = z_exp.reshape(P_TOTAL, M, K).sum(axis=1)
    # block per core into (n_batches, K, 128), zero-padded past PT
    n_batches = math.ceil(PT / 128)
    z_blk = np.zeros((N_CORES, n_batches, K, 128), np_dtype)
    zc = z_sum.astype(np_dtype).T.reshape(K, N_CORES, PT)     # (K, core, pt)
    for c in range(N_CORES):
        for bi in range(n_batches):
            b0 = bi * 128
            bs = min(128, PT - b0)
            z_blk[c, bi, :, 0:bs] = zc[:, c, b0:b0 + bs]

    canvas9 = np.zeros((F, CANVAS, CANVAS), np.float32)
    canvas9[:8, 3:54, 3:54] = W.reshape(8, 51, 51)
    canvas9[8, 3:54, 3:54] = b.reshape(51, 51)
    sw = np.lib.stride_tricks.sliding_window_view(canvas9, (OUT_HW, OUT_HW), axis=(1, 2))
    Wexp = sw.transpose(1, 2, 0, 3, 4).reshape(K, COLS)
    # (2, K, HALF) half-blocked to match the device-side tile loads
    Wexp = np.ascontiguousarray(
        Wexp.reshape(K, 2, HALF).transpose(1, 0, 2), dtype=np_dtype)
    return z_blk, Wexp


def kernel(locs, galaxy_params, galaxy_bool, W_dec, b_dec, _trace=False):
    import ml_dtypes
    from concourse.bass_utils import run_bass_kernel_spmd

    np_dtype = {
        "bf16": ml_dtypes.bfloat16,
        "f32": np.float32,
        "f32r": np.float32,
    }[_DT_NAME]

    z_blk, Wexp = _host_expand(
        locs, galaxy_params, galaxy_bool, W_dec, b_dec, np_dtype)

    nc = _get_program(_DT_NAME)
    in_maps = [
        {
            "zt": z_blk[c],
            "wx": Wexp,
        }
        for c in range(N_CORES)
    ]
    kwargs = {}
    if _trace:
        kwargs["trace"] = True
    res = run_bass_kernel_spmd(nc, in_maps, core_ids=list(range(N_CORES)), **kwargs)

    out = np.concatenate([res.results[c]["out"] for c in range(N_CORES)], axis=0)
    out = out.reshape(P_TOTAL, 1, OUT_HW, OUT_HW)
    if _trace:
        kernel._last_result = res
    return out, out


# revision 7
# speedup vs baseline: 1.4752x; 1.0327x over previous
"""GalaxyTileDecoder on 8 Trainium2 NeuronCores.

The reference pipeline (linear decode -> zero-pad -> gate -> bilinear
grid_sample -> sum over M=2 sources) collapses algebraically: the sample
grid is a pure per-source translation, sampling the padded 53x53 image at
(y, x) = (i + 2.5 - 4*locs[...,0], j + 2.5 - 4*locs[...,1]).  Folding the
integer shift (one-hot over 6 positions per axis), the bilinear weights,
the decoder bias, the galaxy_bool gate, and the M-source sum into an
expanded feature dimension turns the whole forward into one matmul:

    out[p, :] = (sum_par z_exp[p, par, :]) @ W_exp        (K=324)

with W_exp[(a, b, f), (i, j)] = canvas9[f, a+i, b+j] the 6x6 shifted
52x52 windows of the 9 basis images (8 decoder rows + bias) in a 57x57
zero canvas, and z_exp the per-source sparse coefficients
bool * z9[f] * wy[a] * wx[b].  The host computes the tiny coefficient
expansion (~0.002% of FLOPs); the device does the 10000x324x2704 matmul.

Data parallel over the ptile axis: 1250 ptiles per core, no collectives.
"""

import math
import os

import numpy as np

P_TOTAL = 10000
M = 2
N_CORES = 8
PT = P_TOTAL // N_CORES          # ptiles per core
F = 9                            # 8 decoder features + bias
A = 6                            # y-shift positions (-2..3)
B = 6                            # x-shift positions (-2..3)
K = A * B * F                    # 324 expanded features
OUT_HW = 52
COLS = OUT_HW * OUT_HW           # 2704
HALF = COLS // 2                 # 1352
CANVAS = 57

_DT_NAME = os.environ.get("BASS_GAL_DT", "bf16")

_cache = {}


def _build_program(dt_name):
    import concourse.bass as bass  # noqa: F401  (registers engines)
    import concourse.tile as tile
    from concourse import bacc, mybir

    dt_map = {
        "bf16": mybir.dt.bfloat16,
        "f32": mybir.dt.float32,
        "f32r": mybir.dt.float32r,
    }
    DT = dt_map[dt_name]

    n_batches = math.ceil(PT / 128)
    nc = bacc.Bacc(trn_type="TRN2")
    # host-blocked layouts so every DMA reads a fully contiguous DRAM block
    zt = nc.dram_tensor("zt", [K, n_batches * 128], DT, kind="ExternalInput")
    wx = nc.dram_tensor("wx", [2, K, HALF], DT, kind="ExternalInput")
    out = nc.dram_tensor("out", [PT, COLS], mybir.dt.float32, kind="ExternalOutput")

    KCH = [(0, 128), (128, 256), (256, K)]
    SEGS = [(0, 512), (512, 1024), (1024, HALF)]

    with tile.TileContext(nc) as tc:
        with (
            tc.tile_pool(name="w", bufs=1) as wpool,
            tc.tile_pool(name="o", bufs=3) as opool,
            tc.tile_pool(name="ps", bufs=2, space="PSUM") as pspool,
        ):
            # All inputs preloaded upfront on the HWDGE (sync) queue, in the
            # order the first batch consumes them, so the first matmul can
            # start as soon as (w chunk0 half0, z chunk0) land.
            w_tiles = {}
            z_full = []
            for ci, (k0, k1) in enumerate(KCH):
                wt = wpool.tile([k1 - k0, HALF], DT, tag=f"w{ci}_0")
                nc.sync.dma_start(wt[:], wx[0, k0:k1, :])
                w_tiles[ci, 0] = wt
                zb = wpool.tile([k1 - k0, n_batches * 128], DT, tag=f"z{ci}")
                nc.sync.dma_start(zb[:], zt[k0:k1, :])
                z_full.append(zb)
            for ci, (k0, k1) in enumerate(KCH):
                wt = wpool.tile([k1 - k0, HALF], DT, tag=f"w{ci}_1")
                nc.sync.dma_start(wt[:], wx[1, k0:k1, :])
                w_tiles[ci, 1] = wt

            for bi in range(n_batches):
                b0 = bi * 128
                bs = min(128, PT - b0)
                z_b = [z_full[ci][:, bi * 128:bi * 128 + bs] for ci in range(3)]
                for h in range(2):
                    ps = pspool.tile([128, HALF], mybir.dt.float32, tag="ps")
                    for ci in range(len(KCH)):
                        for (s0, s1) in SEGS:
                            nc.tensor.matmul(
                                ps[0:bs, s0:s1],
                                z_b[ci][:, 0:bs],
                                w_tiles[ci, h][:, s0:s1],
                                start=(ci == 0),
                                stop=(ci == len(KCH) - 1),
                            )
                    osb = opool.tile([128, HALF], mybir.dt.float32, tag="osb")
                    nc.vector.tensor_copy(osb[0:bs, :], ps[0:bs, :])
                    nc.sync.dma_start(out[b0:b0 + bs, h * HALF:(h + 1) * HALF],
                                      osb[0:bs, :])
    nc.compile()
    return nc


def _get_program(dt_name):
    if dt_name not in _cache:
        _cache[dt_name] = _build_program(dt_name)
    return _cache[dt_name]


def _host_expand(locs, galaxy_params, galaxy_bool, W_dec, b_dec, np_dtype):
    """Build zt (K, P_TOTAL) parity-summed coefficients and Wexp (K, COLS)."""
    locs = np.asarray(locs, np.float32).reshape(-1, 2)
    params = np.asarray(galaxy_params, np.float32).reshape(-1, 8)
    gbool = np.asarray(galaxy_bool, np.float32).reshape(-1, 1)
    W = np.asarray(W_dec, np.float32)
    b = np.asarray(b_dec, np.float32)
    N = locs.shape[0]

    sy = 2.5 - 4.0 * locs[:, 0]
    sx = 2.5 - 4.0 * locs[:, 1]
    m = np.floor(sy)
    k = np.floor(sx)
    fy = (sy - m).astype(np.float32)
    fx = (sx - k).astype(np.float32)
    m = m.astype(np.int64)
    k = k.astype(np.int64)
    ar = np.arange(N)
    cy = np.zeros((N, A), np.float32)
    cx = np.zeros((N, B), np.float32)
    cy[ar, m + 2] = 1.0 - fy
    cy[ar, m + 3] = fy
    cx[ar, k + 2] = 1.0 - fx
    cx[ar, k + 3] = fx

    z9 = np.concatenate([params, np.ones((N, 1), np.float32)], axis=1) * gbool
    z_exp = (cy[:, :, None, None] * cx[:, None, :, None] * z9[:, None, None, :])
    # sum the M=2 sources of each ptile (matmul is linear in z_exp)
    z_sum = z_exp.reshape(P_TOTAL, M, K).sum(axis=1)
    # per core (K, n_batches*128), zero-padded past PT
    n_batches = math.ceil(PT / 128)
    z_blk = np.zeros((N_CORES, K, n_batches * 128), np_dtype)
    zc = z_sum.astype(np_dtype).T.reshape(K, N_CORES, PT)     # (K, core, pt)
    for c in range(N_CORES):
        z_blk[c, :, 0:PT] = zc[:, c, :]

    canvas9 = np.zeros((F, CANVAS, CANVAS), np.float32)
    canvas9[:8, 3:54, 3:54] = W.reshape(8, 51, 51)
    canvas9[8, 3:54, 3:54] = b.reshape(51, 51)
    sw = np.lib.stride_tricks.sliding_window_view(canvas9, (OUT_HW, OUT_HW), axis=(1, 2))
    Wexp = sw.transpose(1, 2, 0, 3, 4).reshape(K, COLS)
    # (2, K, HALF) half-blocked to match the device-side tile loads
    Wexp = np.ascontiguousarray(
        Wexp.reshape(K, 2, HALF).transpose(1, 0, 2), dtype=np_dtype)
    return z_blk, Wexp


def kernel(locs, galaxy_params, galaxy_bool, W_dec, b_dec, _trace=False):
    import ml_dtypes
    from concourse.bass_utils import run_bass_kernel_spmd

    np_dtype = {
        "bf16": ml_dtypes.bfloat16,
        "f32": np.float32,
        "f32r": np.float32,
    }[_DT_NAME]

    z_blk, Wexp = _host_expand(
        locs, galaxy_params, galaxy_bool, W_dec, b_dec, np_dtype)

    nc = _get_program(_DT_NAME)
    in_maps = [
        {
            "zt": z_blk[c],
            "wx": Wexp,
        }
        for c in range(N_CORES)
    ]
    kwargs = {}
    if _trace:
        kwargs["trace"] = True
    res = run_bass_kernel_spmd(nc, in_maps, core_ids=list(range(N_CORES)), **kwargs)

    out = np.concatenate([res.results[c]["out"] for c in range(N_CORES)], axis=0)
    out = out.reshape(P_TOTAL, 1, OUT_HW, OUT_HW)
    if _trace:
        kernel._last_result = res
    return out, out
